# revision 2
# baseline (speedup 1.0000x reference)
"""Fully-fused fp16 MoE expert FFN (E=8, C=2048, D=1024, F=4096), 8 TRN2 cores.

One expert per core; w1 AND w2 fully SBUF-resident in fp16. v2 changes vs
the serial-DMA baseline:
  - DMA issue is engine-parallel: HWDGE DMAs block the issuing engine for
    the whole transfer (~210 GB/s/engine), so w2 goes on gpsimd (SWDGE,
    otherwise idle), x/w1/out stay on sync in consumption order, and the
    scalar engine keeps only the gelu evictions (no DMA backlog ahead of
    ACTs, which would stall PSUM recycling).
  - Host-side layouts match SBUF order so each w1 column-block / x chunk /
    w2 half is ONE DMA instruction.
  - ~24 dummy matmuls on a memset tile at t=0 keep the PE busy through the
    HAM activity window so real matmuls start at 2.4 GHz, not 1.2.
  - x chunks cn+1 prefetch before chunk cn's mm2 so xt never queues behind
    output DMAs on sync.
"""

import numpy as np

import concourse.bass as bass
import concourse.mybir as mybir
import concourse.tile as tile
from concourse import bacc
from concourse.bass_utils import run_bass_kernel_spmd

E, C, D, F = 8, 2048, 1024, 4096
P = 128
KD = D // P  # 8
MF = F // P  # 32
CN = C // 512  # 4 chunks of 512 tokens
CJ = 4  # 128-token subblocks per chunk
DN = D // 512  # 2
FJ = F // 512  # 8 column blocks of w1
WARMUP = 24

F32 = mybir.dt.float32
F16 = mybir.dt.float16
GELU = mybir.ActivationFunctionType.Gelu_apprx_tanh

_CACHE = {}


def _build():
    nc = bacc.Bacc("TRN2", target_bir_lowering=False, debug=False, num_devices=E)

    # Layouts are pre-transposed on host so every DMA below is a single
    # instruction whose src/dst iteration orders match.
    xh_d = nc.dram_tensor("xh", [CN, P, KD, 512], F16, kind="ExternalInput").ap()
    w1_d = nc.dram_tensor("w1h", [FJ, P, KD, 512], F16, kind="ExternalInput").ap()
    b1_d = nc.dram_tensor("b1t", [P, MF], F32, kind="ExternalInput").ap()
    w2_d = nc.dram_tensor("w2h", [DN, P, MF, 512], F16, kind="ExternalInput").ap()
    out_d = nc.dram_tensor("out", [C, D], F32, kind="ExternalOutput").ap()

    with tile.TileContext(nc) as tc:
        with (
            tc.tile_pool(name="w1f", bufs=1) as w1_pool,
            tc.tile_pool(name="w2f", bufs=1) as w2_pool,
            tc.tile_pool(name="b1", bufs=1) as b1_pool,
            tc.tile_pool(name="xt", bufs=2) as xt_pool,
            tc.tile_pool(name="ht", bufs=1) as ht_pool,
            tc.tile_pool(name="ev", bufs=4) as ev_pool,
            tc.tile_pool(name="wrm", bufs=1) as wrm_pool,
            tc.tile_pool(name="ps1", bufs=4, space="PSUM") as ps1_pool,
            tc.tile_pool(name="ps2", bufs=4, space="PSUM") as ps2_pool,
        ):
            # PE warmup: memset a dummy tile, then a stream of matmuls on it
            # so the HAM clock-gate opens before real data arrives.
            wrm = wrm_pool.tile([P, 512], F16)
            nc.vector.memset(wrm[:], 0.0)
            for _ in range(WARMUP):
                wps = ps2_pool.tile([P, 512], F32, tag="ps2")
                nc.tensor.matmul(wps[:], wrm[:, 0:P], wrm[:], start=True, stop=True)

            b1t = b1_pool.tile([P, MF], F32)
            nc.sync.dma_start(b1t[:], b1_d[:])

            # w2 entirely on gpsimd (SWDGE): issued from t~0, needed only
            # when mm2 starts (~70us). One instruction per 512-col half.
            w2f = w2_pool.tile([P, MF, D], F16)
            for dn in range(DN):
                nc.gpsimd.dma_start(
                    w2f[:, :, bass.ds(dn * 512, 512)], w2_d[dn]
                )

            # chunk 0 activations as per-k instructions so the first mm1
            # group can start k-paced as soon as the first slices land.
            xt0 = xt_pool.tile([P, KD, 512], F16, tag="xt")
            for k in range(KD):
                nc.sync.dma_start(xt0[:, k, :], xh_d[0, :, k, :])

            # w1 column-block jj=0 per-k (startup granularity), the rest as
            # one instruction per column block.
            w1f = w1_pool.tile([P, KD, F], F16)
            for k in range(KD):
                nc.sync.dma_start(w1f[:, k, bass.ds(0, 512)], w1_d[0, :, k, :])
            for jj in range(1, FJ):
                nc.sync.dma_start(
                    w1f[:, :, bass.ds(jj * 512, 512)], w1_d[jj]
                )

            def load_xt(cn):
                t = xt_pool.tile([P, KD, 512], F16, tag="xt")
                nc.sync.dma_start(t[:], xh_d[cn])
                return t

            xt = xt0
            for cn in range(CN):
                ht = ht_pool.tile([P, MF, 512], F16, tag="ht")
                for j in range(MF):
                    ps = ps1_pool.tile([P, 512], F32, tag="ps1")
                    for k in range(KD):
                        nc.tensor.matmul(
                            ps[:],
                            w1f[:, k, bass.ds(j * P, P)],
                            xt[:, k, :],
                            start=(k == 0),
                            stop=(k == KD - 1),
                        )
                    nc.scalar.activation(
                        ht[:, j, :], ps[:], GELU, bias=b1t[:, j : j + 1]
                    )
                # prefetch next chunk now so its load precedes this chunk's
                # output DMAs in the sync engine's program order
                if cn + 1 < CN:
                    xt = load_xt(cn + 1)
                for cj in range(CJ):
                    row = cn * 512 + cj * P
                    for dn in range(DN):
                        ps = ps2_pool.tile([P, 512], F32, tag="ps2")
                        for j in range(MF):
                            nc.tensor.matmul(
                                ps[:],
                                ht[:, j, bass.ds(cj * P, P)],
                                w2f[:, j, bass.ds(dn * 512, 512)],
                                start=(j == 0),
                                stop=(j == MF - 1),
                            )
                        ev = ev_pool.tile([P, 512], F32, tag="ev")
                        nc.vector.tensor_copy(ev[:], ps[:])
                        nc.sync.dma_start(
                            out_d[row : row + P, dn * 512 : (dn + 1) * 512],
                            ev[:],
                        )

    nc.compile()
    return nc


def _get_nc():
    if "nc" not in _CACHE:
        _CACHE["nc"] = _build()
    return _CACHE["nc"]


def _in_map(x_e, w1_e, b1_e, w2_e):
    xT = np.ascontiguousarray(x_e.T).astype(np.float16)  # [D, C]
    xh = np.ascontiguousarray(
        xT.reshape(KD, P, CN, 512).transpose(2, 1, 0, 3)
    )  # [CN, P, KD, 512]
    w1r = w1_e.astype(np.float16).reshape(KD, P, FJ, 512)
    w1h = np.ascontiguousarray(w1r.transpose(2, 1, 0, 3))  # [FJ, P, KD, 512]
    b1t = np.ascontiguousarray(b1_e.reshape(MF, P).T)
    w2r = w2_e.astype(np.float16).reshape(MF, P, DN, 512)
    w2h = np.ascontiguousarray(w2r.transpose(2, 1, 0, 3))  # [DN, P, MF, 512]
    return {"xh": xh, "w1h": w1h, "b1t": b1t, "w2h": w2h}


def kernel(inputs, w1, b1, w2, b2, _trace=False):
    nc = _get_nc()
    x = np.asarray(inputs, dtype=np.float32).reshape(E, C, D)
    in_maps = [
        _in_map(
            x[e],
            np.asarray(w1[e], dtype=np.float32),
            np.asarray(b1[e], dtype=np.float32),
            np.asarray(w2[e], dtype=np.float32),
        )
        for e in range(E)
    ]
    res = run_bass_kernel_spmd(nc, in_maps, list(range(E)), trace=_trace)
    out = np.stack([res.results[e]["out"] for e in range(E)])[None]
    out = out + np.asarray(b2, dtype=np.float32)[None]
    if _trace:
        _CACHE["last_results"] = res
    return out.astype(np.float32)
